# revision 20
# baseline (speedup 1.0000x reference)
"""Multi-head cross-attention Bass/Tile kernel for Trainium2, 8-core SPMD.

Sharding: B=2 batches x 16 heads -> 8 cores, each core owns (batch b, 4
consecutive heads). Everything device-side is per-core independent (no
collectives); host shards inputs / gathers outputs.

Per-core device flow (all matmuls bf16 -> f32 PSUM):
  1. Q/K projected depth-major (qT/kT [256, 2048]) so scoresT needs no
     transposes; V projected token-major with a ones column appended
     (bias handled via an augmented contraction row on the host).
  2. scoresT[k,q] = kT.T @ qT per 128-row k-chunk; ScalarE computes
     expT = exp(scoresT/8) (bf16) straight out of PSUM.
  3. ctxT[d,q] = [v|1].T @ expT accumulated over k-chunks; the ones column
     makes PSUM row 64 the softmax denominators for free.
  4. attn output: PE-transposes expT tiles back to [q,k]; VectorE fuses
     normalize (x recip) + f32 cast + PSUM evacuation; DMA out.
  5. ctx output: PE-transposes ctxT to [q,d]; normalize on evacuation.
"""

import sys

sys.path.insert(0, "/opt/trn_rl_repo")

import numpy as np
import ml_dtypes

import concourse.bacc as bacc
import concourse.tile as tile
import concourse.mybir as mybir
from concourse.bass_utils import run_bass_kernel_spmd
from concourse.masks import make_identity

BF16 = ml_dtypes.bfloat16
NUM_HEADS, B, N, D, HD = 16, 2, 2048, 1024, 64
HPC = 4            # heads per core
DC = HPC * HD      # 256 projection cols per core
NT = N // 128      # 16 token tiles
KC = D // 128      # 8 contraction chunks for q/k
KCV = KC + 1       # +1 augmented chunk carries the v bias

_PROG = None

# scheduling knobs (A/B-tested via the cost-model simulator)
CFG = dict(psum_bufs=2, use_sm=False, expt_extra=0, act_mod=4, aout_bufs=4)


def _build():
    dt = mybir.dt
    nc = bacc.Bacc(None)
    xq_d = nc.dram_tensor("xq", [128, KC, N], dt.bfloat16, kind="ExternalInput")
    xk_d = nc.dram_tensor("xk", [128, KC, N], dt.bfloat16, kind="ExternalInput")
    xv_d = nc.dram_tensor("xv", [128, KCV, N], dt.bfloat16, kind="ExternalInput")
    wq_d = nc.dram_tensor("wq", [128, KC, DC], dt.bfloat16, kind="ExternalInput")
    wk_d = nc.dram_tensor("wk", [128, KC, DC], dt.bfloat16, kind="ExternalInput")
    wv_d = nc.dram_tensor("wv", [128, KCV, DC], dt.bfloat16, kind="ExternalInput")
    bq_d = nc.dram_tensor("bq", [128, 2], dt.float32, kind="ExternalInput")
    bk_d = nc.dram_tensor("bk", [128, 2], dt.float32, kind="ExternalInput")
    attn_d = nc.dram_tensor("attn", [HPC, N, N], dt.float32, kind="ExternalOutput")
    ctx_d = nc.dram_tensor("ctx", [N, DC], dt.float32, kind="ExternalOutput")

    EXP = mybir.ActivationFunctionType.Exp

    with tile.TileContext(nc) as tc:
        with (
            tc.tile_pool(name="const", bufs=1) as const,
            tc.tile_pool(name="weights", bufs=1) as wpool,
            tc.tile_pool(name="proj", bufs=1) as proj,
            tc.tile_pool(name="psum", bufs=CFG["psum_bufs"], space="PSUM") as psum,
            tc.tile_pool(name="psum_sm", bufs=(2 if CFG["use_sm"] else 0) or 1, space="PSUM") as psum_sm,
        ):
            id_bf = const.tile([128, 128], dt.bfloat16)
            make_identity(nc, id_bf)
            id_f32 = const.tile([128, 128], dt.float32)
            make_identity(nc, id_f32)

            wq_sb = wpool.tile([128, KC, DC], dt.bfloat16)
            wk_sb = wpool.tile([128, KC, DC], dt.bfloat16)
            wv_sb = wpool.tile([128, KCV, DC], dt.bfloat16)
            bq_sb = wpool.tile([128, 2], dt.float32)
            bk_sb = wpool.tile([128, 2], dt.float32)
            for sb, dd in (
                (wq_sb, wq_d), (wk_sb, wk_d), (wv_sb, wv_d),
                (bq_sb, bq_d), (bk_sb, bk_d),
            ):
                nc.sync.dma_start(out=sb, in_=dd[:])

            qT_sb = proj.tile([128, 2, N], dt.bfloat16)
            kT_sb = proj.tile([128, 2, N], dt.bfloat16)
            v_sb = proj.tile([128, NT, HPC, HD + 1], dt.bfloat16)
            nc.vector.memset(v_sb[:, :, :, HD:HD + 1], 1.0)

            # ---- q/k projections (m0 first so head-0/1 jobs start early);
            # v projection becomes mergeable units inside pipeline step 0.
            from contextlib import ExitStack
            xt_v_stack = ExitStack()
            xt_v = xt_v_stack.enter_context(tc.tile_pool(name="xt_v", bufs=1))
            xv_sb = xt_v.tile([128, KCV, N], dt.bfloat16)

            def v_unit(t):
                ps = psum.tile([128, 1024], dt.float32, tag="ps")
                for kc in range(KCV):
                    nc.tensor.matmul(
                        ps[:, 0:DC],
                        lhsT=xv_sb[:, kc, t * 128:(t + 1) * 128],
                        rhs=wv_sb[:, kc, :],
                        start=(kc == 0), stop=(kc == KCV - 1),
                    )
                nc.vector.tensor_copy(
                    v_sb[:, t, :, 0:HD],
                    ps[:, 0:DC].rearrange("p (h e) -> p h e", h=HPC),
                )

            v_units = [lambda t=t: v_unit(t) for t in range(NT)]

            with tc.tile_pool(name="xt", bufs=1) as xt:
                xq_sb = xt.tile([128, KC, N], dt.bfloat16)
                xk_sb = xt.tile([128, KC, N], dt.bfloat16)
                for kc in range(KC):
                    nc.sync.dma_start(out=xq_sb[:, kc], in_=xq_d[:, kc])
                    nc.sync.dma_start(out=xk_sb[:, kc], in_=xk_d[:, kc])
                for kc in range(KCV):
                    nc.sync.dma_start(out=xv_sb[:, kc], in_=xv_d[:, kc])

                for m in range(2):
                    for x_sb, w_sb, b_sb, o_sb in (
                        (xq_sb, wq_sb, bq_sb, qT_sb),
                        (xk_sb, wk_sb, bk_sb, kT_sb),
                    ):
                        for nh in range(2):
                            ps = psum.tile([128, 1024], dt.float32, tag="ps")
                            for kc in range(KC):
                                for ns in range(2):
                                    nc.tensor.matmul(
                                        ps[:, ns * 512:(ns + 1) * 512],
                                        lhsT=w_sb[:, kc, m * 128:(m + 1) * 128],
                                        rhs=x_sb[:, kc,
                                                 nh * 1024 + ns * 512:
                                                 nh * 1024 + (ns + 1) * 512],
                                        start=(kc == 0), stop=(kc == KC - 1),
                                    )
                            nc.scalar.add(
                                o_sb[:, m, nh * 1024:(nh + 1) * 1024],
                                ps, b_sb[:, m:m + 1],
                            )

            # ---- attention: 8 jobs (head, q-half), software-pipelined ----
            # Job j = (head, 1024-wide q-half). For each pipeline step we
            # interleave, at emission level, the 16 scores+exp strips of job
            # j with the output-stage units of job j-1 (ctx matmul chunks,
            # softmax-sum plumbing, per-q-tile transpose/normalize/DMA), so
            # every engine has runnable work throughout the step.
            NQH = N // 2          # 1024 q columns per job
            NTH = NQH // 128      # 8 q tiles per job
            JOBS = 2 * HPC
            with (
                tc.tile_pool(name="expT", bufs=2 * NT + CFG["expt_extra"]) as expT_pool,
                tc.tile_pool(name="small", bufs=2) as small,
                tc.tile_pool(name="ctxTp", bufs=2) as ctxTp,
                tc.tile_pool(name="aout", bufs=CFG["aout_bufs"]) as aout,
                tc.tile_pool(name="ctxp", bufs=1) as ctxp,
                tc.tile_pool(name="psum1", bufs=4, space="PSUM") as psum1,
            ):
                ctx_sb = ctxp.tile([128, NT, DC], dt.float32)

                def scores_units(j):
                    h, qh = j // 2, j % 2
                    mstrip, half = h // 2, h % 2
                    qTh = qT_sb[64 * half:64 * half + 64, mstrip, :]
                    kTh = kT_sb[64 * half:64 * half + 64, mstrip, :]
                    expT = [
                        expT_pool.tile([128, NQH], dt.bfloat16, tag="expT",
                                       name=f"expT_{j}_{kc}")
                        for kc in range(NT)
                    ]

                    def strip(kc):
                        ps = psum.tile([128, 1024], dt.float32, tag="ps")
                        for ns in range(2):
                            nc.tensor.matmul(
                                ps[:, ns * 512:(ns + 1) * 512],
                                lhsT=kTh[:, kc * 128:(kc + 1) * 128],
                                rhs=qTh[:, qh * NQH + ns * 512:
                                        qh * NQH + (ns + 1) * 512],
                                start=True, stop=True,
                            )
                        nc.scalar.activation(expT[kc], ps, EXP, scale=0.125)

                    return expT, [lambda kc=kc: strip(kc) for kc in range(NT)]

                def output_units(j, expT):
                    h, qh = j // 2, j % 2
                    # row HD of ctxT carries the softmax denominators
                    ctxT = ctxTp.tile([HD + 1, NQH], dt.float32, tag="ctxT",
                                      name=f"ctxT_{j}")
                    sums = ctxT[HD:HD + 1, :]
                    recip_sb = small.tile([128, NTH], dt.float32, tag="recip",
                                          name=f"recip_{j}")

                    def ctx_chunk(qq):
                        pc = psum1.tile([128, 512], dt.float32, tag="p1")
                        for kc in range(NT):
                            nc.tensor.matmul(
                                pc[0:HD + 1, :],
                                lhsT=v_sb[:, kc, h, :],
                                rhs=expT[kc][:, qq * 512:(qq + 1) * 512],
                                start=(kc == 0), stop=(kc == NT - 1),
                            )
                        nc.vector.tensor_copy(
                            ctxT[:, qq * 512:(qq + 1) * 512], pc[0:HD + 1, :])

                    def sums_plumbing():
                        pr = psum1.tile([128, 512], dt.float32, tag="p1")
                        for i in range(NTH):
                            nc.tensor.transpose(
                                pr[:, i:i + 1],
                                sums[:, i * 128:(i + 1) * 128],
                                id_f32[HD:HD + 1, HD:HD + 1],
                            )
                        sums_sb = small.tile([128, NTH], dt.float32,
                                             tag="sums_sb")
                        nc.vector.tensor_copy(sums_sb, pr[:, 0:NTH])
                        nc.vector.reciprocal(recip_sb, sums_sb)

                    def i_unit(i):
                        iq = qh * NTH + i     # global q tile index
                        at = aout.tile([128, N], dt.float32, tag="at")
                        for kh in range(2):
                            pt = psum1.tile([128, 1024], dt.bfloat16, tag="p1")
                            for kk in range(NT // 2):
                                kc = kh * (NT // 2) + kk
                                nc.tensor.transpose(
                                    pt[:, kk * 128:(kk + 1) * 128],
                                    expT[kc][:, i * 128:(i + 1) * 128],
                                    id_bf,
                                )
                            seg = at[:, kh * 1024:(kh + 1) * 1024]
                            if (2 * i + kh) % CFG["act_mod"] != 1:
                                nc.vector.tensor_scalar_mul(
                                    seg, pt, recip_sb[:, i:i + 1])
                            else:
                                nc.scalar.activation(
                                    seg, pt, mybir.ActivationFunctionType.Copy,
                                    bias=0.0, scale=recip_sb[:, i:i + 1])
                        nc.sync.dma_start(
                            out=attn_d[h, iq * 128:(iq + 1) * 128, :], in_=at)

                        pf = psum1.tile([128, 512], dt.float32, tag="p1")
                        nc.tensor.transpose(
                            pf[:, 0:HD],
                            ctxT[0:HD, i * 128:(i + 1) * 128],
                            id_f32[0:HD, 0:HD],
                        )
                        nc.vector.tensor_scalar_mul(
                            ctx_sb[:, iq, h * HD:(h + 1) * HD],
                            pf[:, 0:HD], recip_sb[:, i:i + 1],
                        )

                    units = [lambda: ctx_chunk(0), lambda: ctx_chunk(1),
                             sums_plumbing]
                    units += [lambda i=i: i_unit(i) for i in range(NTH)]
                    return units

                def merge(a, b):
                    # proportional round-robin merge of two thunk lists
                    out = []
                    ia = ib = 0
                    na, nb = len(a), len(b)
                    while ia < na or ib < nb:
                        if ib >= nb or (ia < na and ia * nb <= ib * na):
                            out.append(a[ia]); ia += 1
                        else:
                            out.append(b[ib]); ib += 1
                    return out

                prev_units = []
                for j in range(JOBS + 1):
                    if j < JOBS:
                        expT, sunits = scores_units(j)
                    else:
                        sunits = []
                    other = prev_units if j > 0 else v_units
                    for u in merge(sunits, other):
                        u()
                    if j < JOBS:
                        prev_units = output_units(j, expT)

                for i in range(NT):
                    nc.sync.dma_start(
                        out=ctx_d[i * 128:(i + 1) * 128, :], in_=ctx_sb[:, i, :])
            xt_v_stack.close()
    nc.finalize()
    return nc


def get_program():
    global _PROG
    if _PROG is None:
        _PROG = _build()
    return _PROG


def _swizzle(a, nchunks):
    """[nchunks*128, M] -> [128, nchunks, M] bf16 (partition-major)."""
    m = a.shape[1]
    return np.ascontiguousarray(
        a.reshape(nchunks, 128, m).transpose(1, 0, 2)).astype(BF16)


def make_in_maps(query, key, value, Wq, bq, Wk, bk, Wv, bv):
    query, key, value = (np.asarray(x, np.float32) for x in (query, key, value))
    Wq, bq, Wk, bk, Wv, bv = (
        np.asarray(x, np.float32) for x in (Wq, bq, Wk, bk, Wv, bv))
    in_maps = []
    xT_cache = {}
    for c in range(8):
        b, hs = c // 4, (c % 4) * HPC
        cs = slice(hs * HD, hs * HD + DC)
        if b not in xT_cache:
            xq = _swizzle(np.ascontiguousarray(query[b].T), KC)
            xk = _swizzle(np.ascontiguousarray(key[b].T), KC)
            vT_aug = np.zeros((KCV * 128, N), np.float32)
            vT_aug[:D] = value[b].T
            vT_aug[D] = 1.0
            xv = _swizzle(vT_aug, KCV)
            xT_cache[b] = (xq, xk, xv)
        xq, xk, xv = xT_cache[b]
        wv_aug = np.zeros((KCV * 128, DC), np.float32)
        wv_aug[:D] = Wv[:, cs]
        wv_aug[D] = bv[cs]
        in_maps.append({
            "xq": xq, "xk": xk, "xv": xv,
            "wq": _swizzle(Wq[:, cs], KC),
            "wk": _swizzle(Wk[:, cs], KC),
            "wv": _swizzle(wv_aug, KCV),
            "bq": np.ascontiguousarray(bq[cs].reshape(2, 128).T),
            "bk": np.ascontiguousarray(bk[cs].reshape(2, 128).T),
        })
    return in_maps


def assemble(results):
    attn = np.empty((B, NUM_HEADS, N, N), np.float32)
    ctx = np.empty((B, N, NUM_HEADS * HD), np.float32)
    for c in range(8):
        b, hs = c // 4, (c % 4) * HPC
        attn[b, hs:hs + HPC] = results[c]["attn"]
        ctx[b][:, hs * HD:hs * HD + DC] = results[c]["ctx"]
    return ctx, attn


def kernel(query, key, value, Wq, bq, Wk, bk, Wv, bv):
    nc = get_program()
    in_maps = make_in_maps(query, key, value, Wq, bq, Wk, bk, Wv, bv)
    res = run_bass_kernel_spmd(nc, in_maps, list(range(8)))
    return assemble(res.results)


# revision 26
# speedup vs baseline: 25.0246x; 25.0246x over previous
"""Multi-head cross-attention Bass/Tile kernel for Trainium2, 8-core SPMD.

Sharding: B=2 batches x 16 heads -> 8 cores, each core owns (batch b, 4
consecutive heads). Everything device-side is per-core independent (no
collectives); host shards inputs / gathers outputs.

Per-core device flow (all matmuls bf16 -> f32 PSUM):
  1. Q/K projected depth-major (qT/kT [256, 2048]) so scoresT needs no
     transposes; V projected token-major with a ones column appended
     (bias handled via an augmented contraction row on the host).
  2. scoresT[k,q] = kT.T @ qT per 128-row k-chunk; ScalarE computes
     expT = exp(scoresT/8) (bf16) straight out of PSUM.
  3. ctxT[d,q] = [v|1].T @ expT accumulated over k-chunks; the ones column
     makes PSUM row 64 the softmax denominators for free.
  4. attn output: PE-transposes expT tiles back to [q,k]; VectorE fuses
     normalize (x recip) + f32 cast + PSUM evacuation; DMA out.
  5. ctx output: PE-transposes ctxT to [q,d]; normalize on evacuation.
"""

import sys

sys.path.insert(0, "/opt/trn_rl_repo")

import numpy as np
import ml_dtypes

import concourse.bacc as bacc
import concourse.tile as tile
import concourse.mybir as mybir
from concourse.bass_utils import run_bass_kernel_spmd
from concourse.masks import make_identity

BF16 = ml_dtypes.bfloat16
NUM_HEADS, B, N, D, HD = 16, 2, 2048, 1024, 64
HPC = 4            # heads per core
DC = HPC * HD      # 256 projection cols per core
NT = N // 128      # 16 token tiles
KC = D // 128      # 8 contraction chunks for q/k
KCV = KC + 1       # +1 augmented chunk carries the v bias

_PROG = {}

# scheduling knobs (A/B-tested via the cost-model simulator)
CFG = dict(psum_bufs=2, use_sm=False, expt_extra=0, act_mod=4, aout_bufs=4)


def _build(repeat=1):
    dt = mybir.dt
    nc = bacc.Bacc(None)
    xq_d = nc.dram_tensor("xq", [128, KC, N], dt.bfloat16, kind="ExternalInput")
    xk_d = nc.dram_tensor("xk", [128, KC, N], dt.bfloat16, kind="ExternalInput")
    xv_d = nc.dram_tensor("xv", [128, KCV, N], dt.bfloat16, kind="ExternalInput")
    wq_d = nc.dram_tensor("wq", [128, KC, DC], dt.bfloat16, kind="ExternalInput")
    wk_d = nc.dram_tensor("wk", [128, KC, DC], dt.bfloat16, kind="ExternalInput")
    wv_d = nc.dram_tensor("wv", [128, KCV, DC], dt.bfloat16, kind="ExternalInput")
    bq_d = nc.dram_tensor("bq", [128, 2], dt.float32, kind="ExternalInput")
    bk_d = nc.dram_tensor("bk", [128, 2], dt.float32, kind="ExternalInput")
    attn_d = nc.dram_tensor("attn", [HPC, N, N], dt.float32, kind="ExternalOutput")
    ctx_d = nc.dram_tensor("ctx", [N, DC], dt.float32, kind="ExternalOutput")

    EXP = mybir.ActivationFunctionType.Exp

    with tile.TileContext(nc) as tc:
      for _rep in range(repeat):
        with (
            tc.tile_pool(name="const", bufs=1) as const,
            tc.tile_pool(name="weights", bufs=1) as wpool,
            tc.tile_pool(name="proj", bufs=1) as proj,
            tc.tile_pool(name="psum", bufs=CFG["psum_bufs"], space="PSUM") as psum,
            tc.tile_pool(name="psum_sm", bufs=(2 if CFG["use_sm"] else 0) or 1, space="PSUM") as psum_sm,
        ):
            id_bf = const.tile([128, 128], dt.bfloat16)
            make_identity(nc, id_bf)
            id_f32 = const.tile([128, 128], dt.float32)
            make_identity(nc, id_f32)

            wq_sb = wpool.tile([128, KC, DC], dt.bfloat16)
            wk_sb = wpool.tile([128, KC, DC], dt.bfloat16)
            wv_sb = wpool.tile([128, KCV, DC], dt.bfloat16)
            bq_sb = wpool.tile([128, 2], dt.float32)
            bk_sb = wpool.tile([128, 2], dt.float32)
            for sb, dd in (
                (wq_sb, wq_d), (wk_sb, wk_d), (wv_sb, wv_d),
                (bq_sb, bq_d), (bk_sb, bk_d),
            ):
                nc.sync.dma_start(out=sb, in_=dd[:])

            # qT/kT stored per-head DUPLICATED on both partition halves
            # ([0:64] and [64:128] hold the same 64 head dims) so the K=64
            # scores matmuls can row-pack two k-strips concurrently.
            qT_sb = proj.tile([128, HPC, N], dt.bfloat16)
            kT_sb = proj.tile([128, HPC, N], dt.bfloat16)
            v_sb = proj.tile([128, NT, HPC, HD + 1], dt.bfloat16)
            nc.vector.memset(v_sb[:, :, :, HD:HD + 1], 1.0)

            # ---- projections (q/k m0 first, then v) ----
            with tc.tile_pool(name="xt", bufs=1) as xt:
                xq_sb = xt.tile([128, KC, N], dt.bfloat16)
                xk_sb = xt.tile([128, KC, N], dt.bfloat16)
                xv_sb = xt.tile([128, KCV, N], dt.bfloat16)
                for kc in range(KC):
                    nc.sync.dma_start(out=xq_sb[:, kc], in_=xq_d[:, kc])
                    nc.sync.dma_start(out=xk_sb[:, kc], in_=xk_d[:, kc])
                for kc in range(KCV):
                    nc.sync.dma_start(out=xv_sb[:, kc], in_=xv_d[:, kc])

                for m in range(2):
                    for x_sb, w_sb, b_sb, o_sb in (
                        (xq_sb, wq_sb, bq_sb, qT_sb),
                        (xk_sb, wk_sb, bk_sb, kT_sb),
                    ):
                        for nh in range(2):
                            ps = psum.tile([128, 1024], dt.float32, tag="ps")
                            for kc in range(KC):
                                for ns in range(2):
                                    nc.tensor.matmul(
                                        ps[:, ns * 512:(ns + 1) * 512],
                                        lhsT=w_sb[:, kc, m * 128:(m + 1) * 128],
                                        rhs=x_sb[:, kc,
                                                 nh * 1024 + ns * 512:
                                                 nh * 1024 + (ns + 1) * 512],
                                        start=(kc == 0), stop=(kc == KC - 1),
                                    )
                            # strip m rows 0:64 belong to head 2m, rows
                            # 64:128 to head 2m+1 — write each into its
                            # per-head duplicated slot
                            nc.scalar.add(
                                o_sb[0:64, 2 * m, nh * 1024:(nh + 1) * 1024],
                                ps[0:64, :], b_sb[0:64, m:m + 1],
                            )
                            nc.scalar.add(
                                o_sb[64:128, 2 * m + 1,
                                     nh * 1024:(nh + 1) * 1024],
                                ps[64:128, :], b_sb[64:128, m:m + 1],
                            )
                # fill the missing partition halves (cross-partition copy
                # needs the DMA/AXI path; engines cannot shift partitions)
                for o_sb in (qT_sb, kT_sb):
                    for m in range(2):
                        nc.sync.dma_start(
                            out=o_sb[64:128, 2 * m, :],
                            in_=o_sb[0:64, 2 * m, :])
                        nc.sync.dma_start(
                            out=o_sb[0:64, 2 * m + 1, :],
                            in_=o_sb[64:128, 2 * m + 1, :])

            # ---- attention: 8 jobs (head, q-half), software-pipelined ----
            # Job j = (head, 1024-wide q-half). For each pipeline step we
            # interleave, at emission level, the 16 scores+exp strips of job
            # j with the output-stage units of job j-1 (ctx matmul chunks,
            # softmax-sum plumbing, per-q-tile transpose/normalize/DMA), so
            # every engine has runnable work throughout the step.
            NQH = N // 2          # 1024 q columns per job
            NTH = NQH // 128      # 8 q tiles per job
            JOBS = 2 * HPC
            with (
                tc.tile_pool(name="expT", bufs=2 * NT + CFG["expt_extra"]) as expT_pool,
                tc.tile_pool(name="small", bufs=2) as small,
                tc.tile_pool(name="ctxTp", bufs=2) as ctxTp,
                tc.tile_pool(name="aout", bufs=CFG["aout_bufs"]) as aout,
                tc.tile_pool(name="ctxp", bufs=1) as ctxp,
                tc.tile_pool(name="psum1", bufs=4, space="PSUM") as psum1,
            ):
                ctx_sb = ctxp.tile([128, NT, DC], dt.float32)

                def scores_units(j):
                    h, qh = j // 2, j % 2
                    expT = [
                        expT_pool.tile([128, NQH], dt.bfloat16, tag="expT",
                                       name=f"expT_{j}_{kc}")
                        for kc in range(NT)
                    ]

                    def strip_pair(kcp):
                        # two k-strips on disjoint PE row-groups (rows 0-63
                        # vs 64-127) — the K=64 matmuls run concurrently
                        kc0, kc1 = 2 * kcp, 2 * kcp + 1
                        ps0 = psum.tile([128, 1024], dt.float32, tag="ps")
                        ps1 = psum.tile([128, 1024], dt.float32, tag="ps")
                        for ns in range(2):
                            qsl = slice(qh * NQH + ns * 512,
                                        qh * NQH + (ns + 1) * 512)
                            nc.tensor.matmul(
                                ps0[:, ns * 512:(ns + 1) * 512],
                                lhsT=kT_sb[0:64, h, kc0 * 128:(kc0 + 1) * 128],
                                rhs=qT_sb[0:64, h, qsl],
                                start=True, stop=True,
                            )
                            nc.tensor.matmul(
                                ps1[:, ns * 512:(ns + 1) * 512],
                                lhsT=kT_sb[64:128, h,
                                           kc1 * 128:(kc1 + 1) * 128],
                                rhs=qT_sb[64:128, h, qsl],
                                start=True, stop=True,
                            )
                        nc.scalar.activation(expT[kc0], ps0, EXP, scale=0.125)
                        nc.scalar.activation(expT[kc1], ps1, EXP, scale=0.125)

                    return expT, [lambda kcp=kcp: strip_pair(kcp)
                                  for kcp in range(NT // 2)]

                def output_units(j, expT):
                    h, qh = j // 2, j % 2
                    # row HD of ctxT carries the softmax denominators
                    ctxT = ctxTp.tile([HD + 1, NQH], dt.float32, tag="ctxT",
                                      name=f"ctxT_{j}")
                    sums = ctxT[HD:HD + 1, :]
                    recip_sb = small.tile([128, NTH], dt.float32, tag="recip",
                                          name=f"recip_{j}")

                    def ctx_chunk(qq):
                        pc = psum1.tile([128, 512], dt.float32, tag="p1")
                        for kc in range(NT):
                            nc.tensor.matmul(
                                pc[0:HD + 1, :],
                                lhsT=v_sb[:, kc, h, :],
                                rhs=expT[kc][:, qq * 512:(qq + 1) * 512],
                                start=(kc == 0), stop=(kc == NT - 1),
                            )
                        nc.vector.tensor_copy(
                            ctxT[:, qq * 512:(qq + 1) * 512], pc[0:HD + 1, :])

                    def sums_plumbing():
                        pr = psum1.tile([128, 512], dt.float32, tag="p1")
                        for i in range(NTH):
                            nc.tensor.transpose(
                                pr[:, i:i + 1],
                                sums[:, i * 128:(i + 1) * 128],
                                id_f32[HD:HD + 1, HD:HD + 1],
                            )
                        sums_sb = small.tile([128, NTH], dt.float32,
                                             tag="sums_sb")
                        nc.vector.tensor_copy(sums_sb, pr[:, 0:NTH])
                        nc.vector.reciprocal(recip_sb, sums_sb)

                    def i_unit(i):
                        iq = qh * NTH + i     # global q tile index
                        at = aout.tile([128, N], dt.float32, tag="at")
                        for kh in range(2):
                            pt = psum1.tile([128, 1024], dt.bfloat16, tag="p1")
                            for kk in range(NT // 2):
                                kc = kh * (NT // 2) + kk
                                nc.tensor.transpose(
                                    pt[:, kk * 128:(kk + 1) * 128],
                                    expT[kc][:, i * 128:(i + 1) * 128],
                                    id_bf,
                                )
                            seg = at[:, kh * 1024:(kh + 1) * 1024]
                            if (2 * i + kh) % CFG["act_mod"] != 1:
                                nc.vector.tensor_scalar_mul(
                                    seg, pt, recip_sb[:, i:i + 1])
                            else:
                                nc.scalar.activation(
                                    seg, pt, mybir.ActivationFunctionType.Copy,
                                    bias=0.0, scale=recip_sb[:, i:i + 1])
                        nc.sync.dma_start(
                            out=attn_d[h, iq * 128:(iq + 1) * 128, :], in_=at)

                        pf = psum1.tile([128, 512], dt.float32, tag="p1")
                        nc.tensor.transpose(
                            pf[:, 0:HD],
                            ctxT[0:HD, i * 128:(i + 1) * 128],
                            id_f32[0:HD, 0:HD],
                        )
                        nc.vector.tensor_scalar_mul(
                            ctx_sb[:, iq, h * HD:(h + 1) * HD],
                            pf[:, 0:HD], recip_sb[:, i:i + 1],
                        )

                    units = [lambda: ctx_chunk(0), lambda: ctx_chunk(1),
                             sums_plumbing]
                    units += [lambda i=i: i_unit(i) for i in range(NTH)]
                    return units

                def merge(a, b):
                    # proportional round-robin merge of two thunk lists
                    out = []
                    ia = ib = 0
                    na, nb = len(a), len(b)
                    while ia < na or ib < nb:
                        if ib >= nb or (ia < na and ia * nb <= ib * na):
                            out.append(a[ia]); ia += 1
                        else:
                            out.append(b[ib]); ib += 1
                    return out

                prev_units = []
                for j in range(JOBS + 1):
                    if j < JOBS:
                        expT, sunits = scores_units(j)
                    else:
                        sunits = []
                    other = prev_units if j > 0 else v_units
                    for u in merge(sunits, other):
                        u()
                    if j < JOBS:
                        prev_units = output_units(j, expT)

                for i in range(NT):
                    nc.sync.dma_start(
                        out=ctx_d[i * 128:(i + 1) * 128, :], in_=ctx_sb[:, i, :])
            xt_v_stack.close()
    nc.finalize()
    return nc


def get_program(repeat=1):
    if repeat not in _PROG:
        _PROG[repeat] = _build(repeat)
    return _PROG[repeat]


def _swizzle(a, nchunks):
    """[nchunks*128, M] -> [128, nchunks, M] bf16 (partition-major)."""
    m = a.shape[1]
    return np.ascontiguousarray(
        a.reshape(nchunks, 128, m).transpose(1, 0, 2)).astype(BF16)


def make_in_maps(query, key, value, Wq, bq, Wk, bk, Wv, bv):
    query, key, value = (np.asarray(x, np.float32) for x in (query, key, value))
    Wq, bq, Wk, bk, Wv, bv = (
        np.asarray(x, np.float32) for x in (Wq, bq, Wk, bk, Wv, bv))
    in_maps = []
    xT_cache = {}
    for c in range(8):
        b, hs = c // 4, (c % 4) * HPC
        cs = slice(hs * HD, hs * HD + DC)
        if b not in xT_cache:
            xq = _swizzle(np.ascontiguousarray(query[b].T), KC)
            xk = _swizzle(np.ascontiguousarray(key[b].T), KC)
            vT_aug = np.zeros((KCV * 128, N), np.float32)
            vT_aug[:D] = value[b].T
            vT_aug[D] = 1.0
            xv = _swizzle(vT_aug, KCV)
            xT_cache[b] = (xq, xk, xv)
        xq, xk, xv = xT_cache[b]
        wv_aug = np.zeros((KCV * 128, DC), np.float32)
        wv_aug[:D] = Wv[:, cs]
        wv_aug[D] = bv[cs]
        in_maps.append({
            "xq": xq, "xk": xk, "xv": xv,
            "wq": _swizzle(Wq[:, cs], KC),
            "wk": _swizzle(Wk[:, cs], KC),
            "wv": _swizzle(wv_aug, KCV),
            "bq": np.ascontiguousarray(bq[cs].reshape(2, 128).T),
            "bk": np.ascontiguousarray(bk[cs].reshape(2, 128).T),
        })
    return in_maps


def assemble(results):
    attn = np.empty((B, NUM_HEADS, N, N), np.float32)
    ctx = np.empty((B, N, NUM_HEADS * HD), np.float32)
    for c in range(8):
        b, hs = c // 4, (c % 4) * HPC
        attn[b, hs:hs + HPC] = results[c]["attn"]
        ctx[b][:, hs * HD:hs * HD + DC] = results[c]["ctx"]
    return ctx, attn


def kernel(query, key, value, Wq, bq, Wk, bk, Wv, bv):
    nc = get_program()
    in_maps = make_in_maps(query, key, value, Wq, bq, Wk, bk, Wv, bv)
    res = run_bass_kernel_spmd(nc, in_maps, list(range(8)))
    return assemble(res.results)
